# revision 1
# baseline (speedup 1.0000x reference)
"""Trainium2 Bass kernel for nn_DenTargetTransformerConv (GNN message passing).

Strategy (graph/data parallel, dst-owner sharding across 8 NeuronCores):
  - Nodes are partitioned by dst-id range; each core owns N/8 nodes and all
    edges whose dst falls in its range. Cores are fully independent (the
    "halo exchange" of src features is materialized host-side as per-section
    compacted gather tables; the device gathers per-edge rows from them).
  - Per core, own nodes are sorted by in-degree and packed into groups of
    128 (SBUF partition dim). Every node in group g gets K[g] edge slots
    (K[g] = max degree in that group position across all cores, so the 8
    cores share one compiled program). Per-edge q||v rows (512B) are
    fetched with bulk dma_gather instructions; scores, edge softmax
    (max-subtraction elided: scores are O(+-5) in f32), and the weighted
    aggregation run on DVE/ACT with free-axis strided reduces. The small
    per-node linears run on PE with the bias folded in via a ones-row.
"""

import numpy as np

import concourse.bacc as bacc
import concourse.bass as bass
import concourse.tile as tile
from concourse import mybir
from concourse.bass_utils import run_bass_kernel_spmd

F32 = mybir.dt.float32
I16 = mybir.dt.int16
AX = mybir.AxisListType
ALU = mybir.AluOpType
ACTF = mybir.ActivationFunctionType

P = 128
NCORES = 8
HD = 64          # H * D
H, D = 4, 16
IN_F = 64

RUNC = 48        # max slot-columns per merged compute run


# ----------------------------------------------------------------- host prep

def _plan(q_src, v_src, feat, src, dst, ncores):
    n = feat.shape[0]
    npc = n // ncores
    ngrp = (npc + P - 1) // P
    grid = ngrp * P
    ndum = grid - npc

    qv = np.concatenate(
        [np.asarray(q_src, np.float32).reshape(n, HD),
         np.asarray(v_src, np.float32).reshape(n, HD)], axis=1)  # [n, 128]

    src = np.asarray(src).astype(np.int64)
    dst = np.asarray(dst).astype(np.int64)
    order = np.argsort(dst, kind="stable")
    dst_s, src_s = dst[order], src[order]
    bounds = np.searchsorted(dst_s, np.arange(ncores + 1) * npc)

    cores = []
    gmax = np.zeros((ncores, ngrp), np.int64)
    for c in range(ncores):
        lo, hi = bounds[c], bounds[c + 1]
        dstL = dst_s[lo:hi] - c * npc          # ascending
        srcL = src_s[lo:hi]
        deg = np.bincount(dstL, minlength=npc)
        starts = np.concatenate([[0], np.cumsum(deg)])
        rank = np.arange(len(dstL)) - starts[dstL]
        perm = np.argsort(deg, kind="stable")  # ascending degree
        pos_of = np.empty(npc, np.int64)
        pos_of[perm] = ndum + np.arange(npc)
        gd = np.zeros(grid, np.int64)
        gd[ndum:] = deg[perm]
        gmax[c] = gd.reshape(ngrp, P).max(1)
        cores.append(dict(dstL=dstL, srcL=srcL, rank=rank, perm=perm,
                          pos_of=pos_of))

    K = np.maximum(gmax.max(0), 1)             # shared per-group slot count
    colbase = np.concatenate([[0], np.cumsum(K)]).astype(np.int64)
    totc = int(colbase[-1])

    # Per-core halo-exchange buffer: each node's K[g] neighbor qv rows are
    # staged contiguously (rows shared by several owned nodes are replicated
    # per consumer), so the device gather needs one descriptor per node.
    # Row layout: colbase[g]*128 + p*K[g] + k  for slot (group g, node p, k).
    per_core = []
    for c in range(ncores):
        cd = cores[c]
        pos_e = cd["pos_of"][cd["dstL"]]       # grid position of each edge
        g_e = pos_e // P
        p_e = pos_e % P
        col_e = colbase[g_e] + cd["rank"]
        tab = np.zeros((totc * P, 2 * HD), np.float32)
        rows = colbase[g_e] * P + p_e * K[g_e] + cd["rank"]
        tab[rows] = qv[cd["srcL"]]
        mask_flat = np.zeros(totc * P, np.float32)
        mask_flat[col_e * P + p_e] = 1.0
        mask_w = mask_flat.reshape(totc, P).T.copy()               # [128, totc]
        per_core.append(dict(tab=tab, mask=mask_w))

    # featT with ones row, per core, grid-permuted: [IN_F+1, grid]
    featTs = []
    feat = np.asarray(feat, np.float32)
    for c in range(ncores):
        ft = np.zeros((IN_F + 1, grid), np.float32)
        ft[IN_F, :] = 1.0
        perm = cores[c]["perm"]
        ft[:IN_F, ndum:] = feat[c * npc + perm].T
        featTs.append(ft)

    # Merge consecutive equal-K groups into runs of <= RUNC slot-columns;
    # all run APs stay within the 3-free-dim ISA limit via (H,D)->HD and
    # (R,K)->RK collapses.
    runs = []
    g = 0
    while g < ngrp:
        k = int(K[g])
        ge = g + 1
        while ge < ngrp and int(K[ge]) == k and (ge - g + 1) * k <= RUNC:
            ge += 1
        runs.append((g, ge, k))
        g = ge
    mrc = max((g1 - g0) * k for (g0, g1, k) in runs)
    rmax = max(g1 - g0 for (g0, g1, k) in runs)

    # identity gather indices for the largest run, wrapped + replicated
    idx_flat = np.arange(P * rmax, dtype=np.int16)
    idx_w = np.tile(idx_flat.reshape(P * rmax // 16, 16).T, (8, 1))

    return dict(n=n, npc=npc, ngrp=ngrp, grid=grid, ndum=ndum, K=K,
                colbase=colbase, totc=totc, runs=runs, mrc=mrc, rmax=rmax,
                idx_w=idx_w, cores=cores, per_core=per_core, featTs=featTs)


# ------------------------------------------------------------- device build

def _build_nc(plan, ncores):
    ngrp, totc, runs = plan["ngrp"], plan["totc"], plan["runs"]
    grid = plan["grid"]
    mrc = plan["mrc"]
    colbase = plan["colbase"]

    nc = bacc.Bacc("TRN2", target_bir_lowering=False, debug=False,
                   num_devices=ncores)

    featT_d = nc.dram_tensor("featT", [IN_F + 1, grid], F32,
                             kind="ExternalInput").ap()
    rmax = plan["rmax"]
    idx_d = nc.dram_tensor("idx", [P, 8 * rmax], I16,
                           kind="ExternalInput").ap()
    mask_d = nc.dram_tensor("mask", [P, totc], F32, kind="ExternalInput").ap()
    tab_d = nc.dram_tensor("tab", [totc * P, 2 * HD], F32,
                           kind="ExternalInput").ap()
    wk_d = nc.dram_tensor("wk", [IN_F + 1, HD], F32, kind="ExternalInput").ap()
    wsk_d = nc.dram_tensor("wsk", [IN_F + 1, HD], F32, kind="ExternalInput").ap()
    # gate weights / ln params / prelu packed on one row:
    # [wg1' (64) | wg2' (64) | bgate (1) | gamma (64) | beta (64) | prelu_a (1)]
    par_d = nc.dram_tensor("par", [1, 258], F32, kind="ExternalInput").ap()
    out_d = nc.dram_tensor("out", [P, ngrp * HD], F32, kind="ExternalOutput").ap()

    with tile.TileContext(nc) as tc:
        with (
            tc.tile_pool(name="singles", bufs=1) as singles,
            tc.tile_pool(name="psum", bufs=4, space="PSUM") as psum,
            tc.tile_pool(name="qvp", bufs=3) as qvp,
            tc.tile_pool(name="scr", bufs=4) as scr,
            tc.tile_pool(name="exs", bufs=4) as exs,
        ):
            # ---- static loads
            featT = singles.tile([IN_F + 1, grid], F32)
            nc.sync.dma_start(out=featT[:], in_=featT_d[:])
            idx_sb = singles.tile([P, 8 * rmax], I16)
            nc.sync.dma_start(out=idx_sb[:], in_=idx_d[:])
            mask_sb = singles.tile([P, totc], F32)
            nc.sync.dma_start(out=mask_sb[:], in_=mask_d[:])
            wk_sb = singles.tile([IN_F + 1, HD], F32)
            nc.sync.dma_start(out=wk_sb[:], in_=wk_d[:])
            wsk_sb = singles.tile([IN_F + 1, HD], F32)
            nc.sync.dma_start(out=wsk_sb[:], in_=wsk_d[:])
            # broadcast params to all partitions (replicating DMA)
            parb = singles.tile([P, 258], F32)
            nc.gpsimd.dma_start(
                out=parb[:],
                in_=bass.AP(tensor=par_d.tensor, offset=par_d.offset,
                            ap=[[0, P], [1, 258]]))
            wg1 = parb[:, 0:64]
            wg2 = parb[:, 64:128]
            bg = parb[:, 128:129]
            gamma = parb[:, 129:193]
            beta = parb[:, 193:257]
            pa = parb[:, 257:258]

            # ---- per-node linears on PE: k = feat@Wk + bk, skip = feat@Wskip + bskip
            k_sb = singles.tile([P, ngrp * HD], F32)
            skip_sb = singles.tile([P, ngrp * HD], F32)
            for g in range(ngrp):
                lhsT = featT[:, g * P:(g + 1) * P]
                pk = psum.tile([P, HD], F32, tag="pk")
                nc.tensor.matmul(out=pk[:], lhsT=lhsT, rhs=wk_sb[:],
                                 start=True, stop=True)
                nc.scalar.activation(out=k_sb[:, g * HD:(g + 1) * HD],
                                     in_=pk[:], func=ACTF.Copy)
                ps = psum.tile([P, HD], F32, tag="pk")
                nc.tensor.matmul(out=ps[:], lhsT=lhsT, rhs=wsk_sb[:],
                                 start=True, stop=True)
                nc.scalar.activation(out=skip_sb[:, g * HD:(g + 1) * HD],
                                     in_=ps[:], func=ACTF.Copy)

            agg_sb = singles.tile([P, ngrp * HD], F32)
            den_sb = singles.tile([P, ngrp * H], F32)
            eps_t = singles.tile([P, 1], F32)
            nc.vector.memset(eps_t[:], 1e-5)

            # ---- edge phase: per merged run (R equal-K groups), gather the
            # nodes' contiguous neighbor blocks (one descriptor per node)
            # and reduce. All APs stay within 3 free dims.
            for (g0r, g1r, K) in runs:
                R = g1r - g0r
                RK = R * K
                r0 = int(colbase[g0r]) * P
                in_ap = tab_d[r0:r0 + RK * P, :].rearrange(
                    "(n k) e -> n (k e)", k=K)
                qv_t = qvp.tile([P, mrc * 2 * HD], F32, tag="qv")
                nc.gpsimd.dma_gather(
                    out_ap=qv_t[:, :RK * 2 * HD].rearrange(
                        "p (c e) -> p c e", e=K * 2 * HD),
                    in_ap=in_ap,
                    idxs_ap=idx_sb[:, :8 * R],
                    num_idxs=P * R,
                    num_idxs_reg=P * R,
                    elem_size=K * 2 * HD,
                    single_packet=False,
                )
                c0g = int(colbase[g0r])
                qv0 = qv_t[:, 0:1]
                pp = qv0.ap[0]

                # score: a[p, rk, h] = sum_d q[p,rk,h,d] * kdst[p,r,h,d]
                q3 = bass.AP(tensor=qv0.tensor, offset=qv0.offset,
                             ap=[pp, [2 * HD * K, R], [2 * HD, K], [1, HD]])
                kk = k_sb[:, g0r * HD:g1r * HD]
                kb = bass.AP(tensor=kk.tensor, offset=kk.offset,
                             ap=[kk.ap[0], [HD, R], [0, K], [1, HD]])
                prod = scr.tile([P, mrc * HD], F32, tag="scr")
                pv = prod[:, :RK * HD]
                p3 = bass.AP(tensor=pv.tensor, offset=pv.offset,
                             ap=[pv.ap[0], [K * HD, R], [HD, K], [1, HD]])
                nc.vector.tensor_tensor(out=p3, in0=q3, in1=kb, op=ALU.mult)

                ex = exs.tile([P, max(mrc, 2 * ngrp // H + 2) * H], F32,
                              tag="ex")
                e3 = bass.AP(tensor=ex[:].tensor, offset=ex[:].offset,
                             ap=[ex[:].ap[0], [H, RK], [1, H]])
                p3r = bass.AP(tensor=pv.tensor, offset=pv.offset,
                              ap=[pv.ap[0], [HD, RK], [D, H], [1, D]])
                nc.vector.tensor_reduce(out=e3, in_=p3r, axis=AX.X,
                                        op=ALU.add)
                # ex = exp(a/4) * mask
                exf = ex[:, :RK * H]
                nc.scalar.activation(out=exf, in_=exf, func=ACTF.Exp,
                                     scale=0.25)
                mm = mask_sb[:, c0g:c0g + RK]
                mb = bass.AP(tensor=mm.tensor, offset=mm.offset,
                             ap=[mm.ap[0], [1, RK], [0, H]])
                e2 = bass.AP(tensor=exf.tensor, offset=exf.offset,
                             ap=[exf.ap[0], [H, RK], [1, H]])
                nc.vector.tensor_tensor(out=e2, in0=e2, in1=mb, op=ALU.mult)
                # denom[p, r, h] = sum_k ex
                dd = den_sb[:, g0r * H:g1r * H]
                e4 = bass.AP(tensor=exf.tensor, offset=exf.offset,
                             ap=[exf.ap[0], [K * H, R], [1, H], [H, K]])
                nc.vector.tensor_reduce(out=dd, in_=e4, axis=AX.X,
                                        op=ALU.add)
                # w[p, rk, h, d] = v * ex ; agg[p, r, hd] = sum_k w
                v3 = bass.AP(tensor=qv0.tensor, offset=qv0.offset + HD,
                             ap=[pp, [2 * HD, RK], [D, H], [1, D]])
                eb = bass.AP(tensor=exf.tensor, offset=exf.offset,
                             ap=[exf.ap[0], [H, RK], [1, H], [0, D]])
                w_t = scr.tile([P, mrc * HD], F32, tag="scr")
                wv = w_t[:, :RK * HD]
                w3 = bass.AP(tensor=wv.tensor, offset=wv.offset,
                             ap=[wv.ap[0], [HD, RK], [D, H], [1, D]])
                nc.vector.tensor_tensor(out=w3, in0=v3, in1=eb, op=ALU.mult)
                aa = agg_sb[:, g0r * HD:g1r * HD]
                wr = bass.AP(tensor=wv.tensor, offset=wv.offset,
                             ap=[wv.ap[0], [K * HD, R], [1, HD], [HD, K]])
                nc.vector.tensor_reduce(out=aa, in_=wr, axis=AX.X,
                                        op=ALU.add)

            # ---- node phase
            NG = ngrp
            # dinv = 1 / (den + 1e-9)
            nc.vector.tensor_scalar(out=den_sb[:], in0=den_sb[:],
                                    scalar1=1e-9, scalar2=None, op0=ALU.add)
            nc.vector.reciprocal(out=den_sb[:], in_=den_sb[:])
            # rst = agg * dinv (broadcast over d)
            rst = agg_sb
            din = den_sb[:]
            dinb = bass.AP(tensor=din.tensor, offset=din.offset,
                           ap=[din.ap[0], [1, NG * H], [0, D]])
            r3 = rst[:].rearrange("p (c d) -> p c d", d=D)
            nc.vector.tensor_tensor(out=r3, in0=r3, in1=dinb, op=ALU.mult)

            # gate logits
            z_t = singles.tile([P, ngrp * HD], F32)
            gl = exs.tile([P, max(mrc, 2 * ngrp // H + 2) * H], F32, tag="ex")
            wg1b = bass.AP(tensor=wg1.tensor, offset=wg1.offset,
                           ap=[wg1.ap[0], [0, NG], [1, HD]])
            wg2b = bass.AP(tensor=wg2.tensor, offset=wg2.offset,
                           ap=[wg2.ap[0], [0, NG], [1, HD]])
            zv = z_t[:, :NG * HD].rearrange("p (c f) -> p c f", f=HD)
            nc.vector.tensor_tensor(out=zv, in0=skip_sb[:].rearrange(
                "p (c f) -> p c f", f=HD), in1=wg1b, op=ALU.mult)
            nc.vector.tensor_reduce(out=gl[:, 0:NG], in_=zv, axis=AX.X,
                                    op=ALU.add)
            nc.gpsimd.tensor_tensor(out=zv, in0=rst[:].rearrange(
                "p (c f) -> p c f", f=HD), in1=wg2b, op=ALU.mult)
            nc.vector.tensor_reduce(out=gl[:, NG:2 * NG], in_=zv, axis=AX.X,
                                    op=ALU.add)
            nc.vector.tensor_tensor(out=gl[:, 0:NG], in0=gl[:, 0:NG],
                                    in1=gl[:, NG:2 * NG], op=ALU.add)
            nc.scalar.activation(out=gl[:, 0:NG], in_=gl[:, 0:NG],
                                 func=ACTF.Sigmoid, bias=bg)
            # rst = rst + gate * (skip - rst)
            dif = z_t[:, :NG * HD]
            nc.vector.tensor_tensor(out=dif, in0=skip_sb[:], in1=rst[:],
                                    op=ALU.subtract)
            gv = gl[:, 0:NG]
            gb_ = bass.AP(tensor=gv.tensor, offset=gv.offset,
                          ap=[gv.ap[0], [1, NG], [0, HD]])
            d3v = dif.rearrange("p (c f) -> p c f", f=HD)
            nc.vector.tensor_tensor(out=d3v, in0=d3v, in1=gb_, op=ALU.mult)
            nc.gpsimd.tensor_tensor(out=rst[:], in0=rst[:], in1=dif,
                                    op=ALU.add)

            # LayerNorm
            mu = exs.tile([P, max(mrc, 2 * ngrp // H + 2) * H], F32, tag="ex")
            r3f = rst[:].rearrange("p (c f) -> p c f", f=HD)
            nc.vector.tensor_reduce(out=mu[:, 0:NG], in_=r3f, axis=AX.X,
                                    op=ALU.add)
            nc.vector.tensor_scalar(out=mu[:, 0:NG], in0=mu[:, 0:NG],
                                    scalar1=1.0 / HD, scalar2=None,
                                    op0=ALU.mult)
            mub = bass.AP(tensor=mu[:].tensor, offset=mu[:].offset,
                          ap=[mu[:].ap[0], [1, NG], [0, HD]])
            nc.vector.tensor_tensor(out=r3f, in0=r3f, in1=mub, op=ALU.subtract)
            sq = z_t[:, :NG * HD]
            nc.gpsimd.tensor_tensor(out=sq, in0=rst[:], in1=rst[:],
                                    op=ALU.mult)
            vs = mu[:, NG:2 * NG]
            nc.vector.tensor_reduce(out=vs, in_=sq.rearrange(
                "p (c f) -> p c f", f=HD), axis=AX.X, op=ALU.add)
            nc.scalar.activation(out=vs, in_=vs, func=ACTF.Sqrt,
                                 scale=1.0 / HD, bias=eps_t[:])
            nc.vector.reciprocal(out=vs, in_=vs)
            vsb = bass.AP(tensor=vs.tensor, offset=vs.offset,
                          ap=[vs.ap[0], [1, NG], [0, HD]])
            nc.vector.tensor_tensor(out=r3f, in0=r3f, in1=vsb, op=ALU.mult)
            gammab = bass.AP(tensor=gamma.tensor, offset=gamma.offset,
                             ap=[gamma.ap[0], [0, NG], [1, HD]])
            nc.vector.tensor_tensor(out=r3f, in0=r3f, in1=gammab, op=ALU.mult)
            betab = bass.AP(tensor=beta.tensor, offset=beta.offset,
                            ap=[beta.ap[0], [0, NG], [1, HD]])
            nc.gpsimd.tensor_tensor(out=r3f, in0=r3f, in1=betab, op=ALU.add)
            # prelu: max(x,0) + a*min(x,0)
            pos = z_t[:, :NG * HD]
            nc.vector.tensor_scalar(out=pos, in0=rst[:], scalar1=0.0,
                                    scalar2=None, op0=ALU.max)
            nc.vector.tensor_scalar(out=rst[:], in0=rst[:], scalar1=0.0,
                                    scalar2=None, op0=ALU.min)
            nc.vector.scalar_tensor_tensor(out=rst[:], in0=rst[:], scalar=pa,
                                           in1=pos, op0=ALU.mult, op1=ALU.add)
            nc.sync.dma_start(out=out_d[:], in_=rst[:])

    nc.compile()
    return nc


# ------------------------------------------------------------------- driver

_CACHE = {}


def _get_nc(plan, ncores):
    key = (tuple(plan["K"].tolist()), plan["grid"], plan["totc"], ncores)
    if key not in _CACHE:
        _CACHE[key] = _build_nc(plan, ncores)
    return _CACHE[key]


def _make_inmaps(plan, params, ncores):
    (Wk, bk, Wskip, bskip, Wgate, bgate, ln_gamma, ln_beta, prelu_a) = params
    wk = np.concatenate([np.asarray(Wk, np.float32),
                         np.asarray(bk, np.float32).reshape(1, HD)])
    wsk = np.concatenate([np.asarray(Wskip, np.float32),
                          np.asarray(bskip, np.float32).reshape(1, HD)])
    wg = np.asarray(Wgate, np.float32).reshape(3 * HD)
    par = np.zeros((1, 258), np.float32)
    par[0, 0:64] = wg[0:64] + wg[128:192]        # acts on skip
    par[0, 64:128] = wg[64:128] - wg[128:192]    # acts on rst
    par[0, 128] = np.float32(np.asarray(bgate).reshape(-1)[0])
    par[0, 129:193] = np.asarray(ln_gamma, np.float32)
    par[0, 193:257] = np.asarray(ln_beta, np.float32)
    par[0, 257] = np.float32(np.asarray(prelu_a).reshape(-1)[0])

    in_maps = []
    for c in range(ncores):
        pc = plan["per_core"][c]
        m = dict(featT=plan["featTs"][c], idx=plan["idx_w"], mask=pc["mask"],
                 tab=pc["tab"], wk=wk, wsk=wsk, par=par)
        in_maps.append(m)
    return in_maps


def run(q_src, v_src, feat, src, dst, Wk, bk, Wskip, bskip, Wgate, bgate,
        ln_gamma, ln_beta, prelu_a, ncores=NCORES, trace=False):
    plan = _plan(q_src, v_src, feat, src, dst, ncores)
    nc = _get_nc(plan, ncores)
    in_maps = _make_inmaps(
        plan, (Wk, bk, Wskip, bskip, Wgate, bgate, ln_gamma, ln_beta, prelu_a),
        ncores)
    res = run_bass_kernel_spmd(nc, in_maps, core_ids=list(range(ncores)),
                               trace=trace)
    n, npc, ngrp = plan["n"], plan["npc"], plan["ngrp"]
    out = np.empty((n, HD), np.float32)
    for c in range(ncores):
        r = res.results[c]["out"]                          # [128, ngrp*64]
        arr = r.reshape(P, ngrp, HD).transpose(1, 0, 2).reshape(-1, HD)
        out[c * npc + plan["cores"][c]["perm"]] = arr[plan["ndum"]:plan["ndum"] + npc]
    return out, res, plan, in_maps, nc


def kernel(**inputs):
    out, _, _, _, _ = run(**inputs)
    return out



# revision 5
# speedup vs baseline: 1.9583x; 1.9583x over previous
"""Trainium2 Bass kernel for nn_DenTargetTransformerConv (GNN message passing).

Strategy (graph/data parallel, dst-owner sharding across 8 NeuronCores):
  - Nodes are partitioned by dst-id range; each core owns N/8 nodes and all
    edges whose dst falls in its range. Cores are fully independent (the
    "halo exchange" of src features is materialized host-side as per-core
    compacted per-edge tables; the device streams them contiguously).
  - Per core, own nodes are sorted by in-degree and packed into groups of
    128 (SBUF partition dim). Every node in group g gets K[g] edge slots
    (K[g] = max degree in that group position across all cores, so the 8
    cores share one compiled program).
  - All per-edge data lives in fp16 with (d, h)-interleaved head layout so
    every DVE op is a dense step-1 16-bit op (2x perf mode). The edge table
    is stored partition-major in DRAM, so each run is one big contiguous
    dma_start per partition (no gather descriptors).
  - Reductions avoid tensor_reduce (always 1x on DVE) where they are large:
    the D-reduction of scores and the K-reduction of the weighted values are
    log2 trees of 2x tensor_tensor adds. exp/sigmoid/sqrt/PReLU and all
    scalar broadcasts run on the otherwise-idle ACT engine; the node linears
    run on PE with bias folded via a ones-row, 4 groups batched per PSUM
    bank.
"""

import numpy as np

import concourse.bacc as bacc
import concourse.bass as bass
import concourse.tile as tile
from concourse import mybir
from concourse.bass_utils import run_bass_kernel_spmd

F32 = mybir.dt.float32
F16 = mybir.dt.float16
AX = mybir.AxisListType
ALU = mybir.AluOpType
ACTF = mybir.ActivationFunctionType

P = 128
NCORES = 8
HD = 64          # H * D
H, D = 4, 16
IN_F = 64

RUNC = 96        # max slot-columns per merged compute run


def _perm_dh(m):
    """Permute the last hd axis from (h, d) to (d, h) order."""
    s = m.shape[:-1]
    return m.reshape(*s, H, D).swapaxes(-1, -2).reshape(*s, HD)


# ----------------------------------------------------------------- host prep

def _plan(q_src, v_src, feat, src, dst, ncores):
    n = feat.shape[0]
    npc = n // ncores
    ngrp = (npc + P - 1) // P
    grid = ngrp * P
    ndum = grid - npc

    q2 = _perm_dh(np.asarray(q_src, np.float32).reshape(n, HD))
    v2 = _perm_dh(np.asarray(v_src, np.float32).reshape(n, HD))
    qv = np.concatenate([q2, v2], axis=1).astype(np.float16)   # [n, 128]

    src = np.asarray(src).astype(np.int64)
    dst = np.asarray(dst).astype(np.int64)
    order = np.argsort(dst, kind="stable")
    dst_s, src_s = dst[order], src[order]
    bounds = np.searchsorted(dst_s, np.arange(ncores + 1) * npc)

    cores = []
    gmax = np.zeros((ncores, ngrp), np.int64)
    for c in range(ncores):
        lo, hi = bounds[c], bounds[c + 1]
        dstL = dst_s[lo:hi] - c * npc          # ascending
        srcL = src_s[lo:hi]
        deg = np.bincount(dstL, minlength=npc)
        starts = np.concatenate([[0], np.cumsum(deg)])
        rank = np.arange(len(dstL)) - starts[dstL]
        perm = np.argsort(deg, kind="stable")  # ascending degree
        pos_of = np.empty(npc, np.int64)
        pos_of[perm] = ndum + np.arange(npc)
        gd = np.zeros(grid, np.int64)
        gd[ndum:] = deg[perm]
        gmax[c] = gd.reshape(ngrp, P).max(1)
        cores.append(dict(dstL=dstL, srcL=srcL, rank=rank, perm=perm,
                          pos_of=pos_of))

    K = np.maximum(gmax.max(0), 2)             # shared per-group slot count
    colbase = np.concatenate([[0], np.cumsum(K)]).astype(np.int64)
    totc = int(colbase[-1])

    # Per-core edge tables, partition-major: tab[p, col, :] is the qv row of
    # the edge in slot (group g, partition p, rank k), col = colbase[g] + k.
    per_core = []
    for c in range(ncores):
        cd = cores[c]
        pos_e = cd["pos_of"][cd["dstL"]]       # grid position of each edge
        g_e = pos_e // P
        p_e = pos_e % P
        col_e = colbase[g_e] + cd["rank"]
        tab = np.zeros((P, totc, 2 * HD), np.float16)
        tab[p_e, col_e] = qv[cd["srcL"]]
        m2 = np.zeros((P, totc), np.float16)
        m2[p_e, col_e] = 1.0
        mask_h = np.repeat(m2, H, axis=1)                      # [P, totc*H]
        per_core.append(dict(tab=tab.reshape(P, totc * 2 * HD), mask=mask_h))

    # featT with ones row, per core, grid-permuted: [IN_F+1, grid] fp16
    featTs = []
    feat = np.asarray(feat, np.float32)
    for c in range(ncores):
        ft = np.zeros((IN_F + 1, grid), np.float16)
        ft[IN_F, :] = 1.0
        perm = cores[c]["perm"]
        ft[:IN_F, ndum:] = feat[c * npc + perm].T.astype(np.float16)
        featTs.append(ft)

    # Merge consecutive equal-K groups into runs of <= RUNC slot-columns.
    runs = []
    g = 0
    while g < ngrp:
        k = int(K[g])
        ge = g + 1
        while ge < ngrp and int(K[ge]) == k and (ge - g + 1) * k <= RUNC:
            ge += 1
        runs.append((g, ge, k))
        g = ge

    return dict(n=n, npc=npc, ngrp=ngrp, grid=grid, ndum=ndum, K=K,
                colbase=colbase, totc=totc, runs=runs,
                cores=cores, per_core=per_core, featTs=featTs)


# ------------------------------------------------------------- device build

def _ap(view, off, dims):
    """AP over a tile view's buffer: partition dim kept, free dims replaced."""
    return bass.AP(tensor=view.tensor, offset=view.offset + off,
                   ap=[view.ap[0]] + dims)


def _build_nc(plan, ncores):
    ngrp, totc, runs = plan["ngrp"], plan["totc"], plan["runs"]
    grid = plan["grid"]
    colbase = plan["colbase"]
    NG = ngrp

    nc = bacc.Bacc("TRN2", target_bir_lowering=False, debug=False,
                   num_devices=ncores)

    featT_d = nc.dram_tensor("featT", [IN_F + 1, grid], F16,
                             kind="ExternalInput").ap()
    mask_d = nc.dram_tensor("mask", [P, totc * H], F16,
                            kind="ExternalInput").ap()
    tab_d = nc.dram_tensor("tab", [P, totc * 2 * HD], F16,
                           kind="ExternalInput").ap()
    # combined node linear weights: cols 0:64 = Wskip|bskip, 64:128 = Wk|bk
    wks_d = nc.dram_tensor("wks", [IN_F + 1, 2 * HD], F16,
                           kind="ExternalInput").ap()
    # fp16 params: [wg_skip (64) | wg_rst (64) | gamma (64) | beta (64)]
    par16_d = nc.dram_tensor("par16", [1, 4 * HD], F16,
                             kind="ExternalInput").ap()
    # fp32 params: [bgate, prelu_a, ln_eps, 0]
    par32_d = nc.dram_tensor("par32", [1, 4], F32, kind="ExternalInput").ap()
    out_d = nc.dram_tensor("out", [P, ngrp * 2 * HD], F16,
                           kind="ExternalOutput").ap()

    with tile.TileContext(nc) as tc:
        with (
            tc.tile_pool(name="singles", bufs=1) as singles,
            tc.tile_pool(name="psum", bufs=2, space="PSUM") as psum,
            tc.tile_pool(name="qvp", bufs=2) as qvp,
            tc.tile_pool(name="scr", bufs=4) as scr,
            tc.tile_pool(name="t1p", bufs=2) as t1p,
            tc.tile_pool(name="t2p", bufs=2) as t2p,
            tc.tile_pool(name="t3p", bufs=2) as t3p,
            tc.tile_pool(name="exp", bufs=4) as exsp,
            tc.tile_pool(name="smal", bufs=6) as smal,
        ):
            # ---- static loads
            featT = singles.tile([IN_F + 1, grid], F16)
            nc.sync.dma_start(out=featT[:], in_=featT_d[:])
            wks_sb = singles.tile([IN_F + 1, 2 * HD], F16)
            nc.sync.dma_start(out=wks_sb[:], in_=wks_d[:])
            mask_sb = singles.tile([P, totc * H], F16)
            nc.sync.dma_start(out=mask_sb[:], in_=mask_d[:])
            p16 = singles.tile([P, 4 * HD], F16)
            nc.gpsimd.dma_start(
                out=p16[:],
                in_=bass.AP(tensor=par16_d.tensor, offset=par16_d.offset,
                            ap=[[0, P], [1, 4 * HD]]))
            p32 = singles.tile([P, 4], F32)
            nc.gpsimd.dma_start(
                out=p32[:],
                in_=bass.AP(tensor=par32_d.tensor, offset=par32_d.offset,
                            ap=[[0, P], [1, 4]]))
            bg = p32[:, 0:1]
            pa = p32[:, 1:2]
            eps_t = p32[:, 2:3]
            nbias = p32[:, 3:4]          # -2.0 shift for exp

            # ksk: per group g, cols [g*128, g*128+64) = skip,
            # [g*128+64, (g+1)*128) = k16 (later overwritten by rst).
            ksk = singles.tile([P, NG * 2 * HD], F16)
            den = singles.tile([P, NG * H], F32)

            # ---- node linears on PE, 4 groups per PSUM bank
            g = 0
            while g < NG:
                nb = min(4, NG - g)
                pk = psum.tile([P, 512], F32, tag="mm")
                for j in range(nb):
                    nc.tensor.matmul(out=pk[:, j * 128:(j + 1) * 128],
                                     lhsT=featT[:, (g + j) * P:(g + j + 1) * P],
                                     rhs=wks_sb[:],
                                     start=True, stop=True)
                nc.scalar.activation(out=ksk[:, g * 128:(g + nb) * 128],
                                     in_=pk[:, :nb * 128], func=ACTF.Copy)
                g += nb

            # ---- edge phase
            for (g0, g1, K) in runs:
                R = g1 - g0
                RK = R * K
                c0 = int(colbase[g0])
                qv_t = qvp.tile([P, RUNC * 2 * HD], F16, tag="qv")
                nc.sync.dma_start(out=qv_t[:, :RK * 2 * HD],
                                  in_=tab_d[:, c0 * 2 * HD:(c0 + RK) * 2 * HD])
                qv0 = qv_t[:, 0:1]

                # prod[p, (r,k), dh] = q * k_dst  (k16 bcast over k slots)
                prod = scr.tile([P, RUNC * HD], F16, tag="scr")
                pr0 = prod[:, 0:1]
                nc.vector.tensor_tensor(
                    out=_ap(pr0, 0, [[HD * K, R], [HD, K], [1, HD]]),
                    in0=_ap(qv0, 0, [[2 * HD * K, R], [2 * HD, K], [1, HD]]),
                    in1=_ap(ksk[:, 0:1], g0 * 2 * HD + HD,
                            [[2 * HD, R], [0, K], [1, HD]]),
                    op=ALU.mult)

                # score tree over d: prod [p, rk, d16, h] -> a [p, rk, h]
                t1 = t1p.tile([P, RUNC * 32], F16, tag="t1")
                nc.vector.tensor_tensor(
                    out=_ap(t1[:, 0:1], 0, [[32, RK], [H, 8], [1, H]]),
                    in0=_ap(pr0, 0, [[HD, RK], [H, 8], [1, H]]),
                    in1=_ap(pr0, 32, [[HD, RK], [H, 8], [1, H]]),
                    op=ALU.add)
                t2 = t2p.tile([P, RUNC * 16], F16, tag="t2")
                nc.vector.tensor_tensor(
                    out=_ap(t2[:, 0:1], 0, [[16, RK], [H, 4], [1, H]]),
                    in0=_ap(t1[:, 0:1], 0, [[32, RK], [H, 4], [1, H]]),
                    in1=_ap(t1[:, 0:1], 16, [[32, RK], [H, 4], [1, H]]),
                    op=ALU.add)
                t3 = t3p.tile([P, RUNC * 8], F16, tag="t3")
                nc.vector.tensor_tensor(
                    out=_ap(t3[:, 0:1], 0, [[8, RK], [H, 2], [1, H]]),
                    in0=_ap(t2[:, 0:1], 0, [[16, RK], [H, 2], [1, H]]),
                    in1=_ap(t2[:, 0:1], 8, [[16, RK], [H, 2], [1, H]]),
                    op=ALU.add)
                ex = exsp.tile([P, RUNC * H], F16, tag="ex")
                nc.vector.tensor_tensor(
                    out=_ap(ex[:, 0:1], 0, [[H, RK], [1, H]]),
                    in0=_ap(t3[:, 0:1], 0, [[2 * H, RK], [1, H]]),
                    in1=_ap(t3[:, 0:1], H, [[2 * H, RK], [1, H]]),
                    op=ALU.add)

                # ex = exp(a/4 - 2); padded slots forced to 0 by the mask
                exf = ex[:, :RK * H]
                nc.scalar.activation(out=exf, in_=exf, func=ACTF.Exp,
                                     scale=0.25, bias=nbias)
                nc.vector.tensor_tensor(
                    out=exf, in0=exf, in1=mask_sb[:, c0 * H:(c0 + RK) * H],
                    op=ALU.mult)

                # denom[p, r, h] = sum_k ex
                nc.vector.tensor_reduce(
                    out=_ap(den[:, 0:1], g0 * H, [[H, R], [1, H]]),
                    in_=_ap(ex[:, 0:1], 0, [[K * H, R], [1, H], [H, K]]),
                    axis=AX.X, op=ALU.add)

                # w[p, rk, d, h] = v * ex (bcast over d)
                w_t = scr.tile([P, RUNC * HD], F16, tag="scr")
                w0 = w_t[:, 0:1]
                nc.vector.tensor_tensor(
                    out=_ap(w0, 0, [[HD, RK], [H, D], [1, H]]),
                    in0=_ap(qv0, HD, [[2 * HD, RK], [H, D], [1, H]]),
                    in1=_ap(ex[:, 0:1], 0, [[H, RK], [0, D], [1, H]]),
                    op=ALU.mult)

                # agg tree over k -> rst slot of ksk (fp16)
                klen = K
                while klen > 2:
                    h1 = klen // 2
                    nc.vector.tensor_tensor(
                        out=_ap(w0, 0, [[K * HD, R], [HD, h1], [1, HD]]),
                        in0=_ap(w0, 0, [[K * HD, R], [HD, h1], [1, HD]]),
                        in1=_ap(w0, (klen - h1) * HD,
                                [[K * HD, R], [HD, h1], [1, HD]]),
                        op=ALU.add)
                    klen = h1 + (klen & 1)
                nc.vector.tensor_tensor(
                    out=_ap(ksk[:, 0:1], g0 * 2 * HD + HD,
                            [[2 * HD, R], [1, HD]]),
                    in0=_ap(w0, 0, [[K * HD, R], [1, HD]]),
                    in1=_ap(w0, HD, [[K * HD, R], [1, HD]]),
                    op=ALU.add)

            # ---- node phase
            kv = ksk[:, 0:1]
            # dinv = 1 / (den + 1e-9), as fp16 bcast
            nc.vector.tensor_scalar(out=den[:], in0=den[:], scalar1=1e-9,
                                    scalar2=None, op0=ALU.add)
            nc.vector.reciprocal(out=den[:], in_=den[:])
            d16 = smal.tile([P, NG * H], F16, tag="sm")
            nc.scalar.activation(out=d16[:], in_=den[:], func=ACTF.Copy)
            # rst = agg * dinv
            rst3 = _ap(kv, HD, [[2 * HD, NG], [H, D], [1, H]])
            nc.vector.tensor_tensor(
                out=rst3, in0=rst3,
                in1=_ap(d16[:, 0:1], 0, [[H, NG], [0, D], [1, H]]),
                op=ALU.mult)

            # gate logit: z = sum over 128 of [skip|rst] * [wgs|wgr]
            zt = qvp.tile([P, RUNC * 2 * HD], F16, tag="qv")
            nc.vector.tensor_tensor(
                out=_ap(zt[:, 0:1], 0, [[2 * HD, NG], [1, 2 * HD]]),
                in0=_ap(kv, 0, [[2 * HD, NG], [1, 2 * HD]]),
                in1=_ap(p16[:, 0:1], 0, [[0, NG], [1, 2 * HD]]),
                op=ALU.mult)
            gl = smal.tile([P, NG], F32, tag="sm")
            nc.vector.tensor_reduce(
                out=gl[:],
                in_=_ap(zt[:, 0:1], 0, [[2 * HD, NG], [1, 2 * HD]]),
                axis=AX.X, op=ALU.add)
            g16 = smal.tile([P, NG], F16, tag="sm")
            nc.scalar.activation(out=g16[:], in_=gl[:], func=ACTF.Sigmoid,
                                 bias=bg)
            gb = smal.tile([P, NG * HD], F16, tag="sm")
            nc.scalar.activation(
                out=gb[:], in_=_ap(g16[:, 0:1], 0, [[1, NG], [0, HD]]),
                func=ACTF.Copy)
            # rst += gate * (skip - rst)
            dif = smal.tile([P, NG * HD], F16, tag="sm")
            nc.vector.tensor_tensor(
                out=dif[:].rearrange("p (c f) -> p c f", f=HD),
                in0=_ap(kv, 0, [[2 * HD, NG], [1, HD]]),
                in1=_ap(kv, HD, [[2 * HD, NG], [1, HD]]),
                op=ALU.subtract)
            nc.vector.tensor_tensor(out=dif[:], in0=dif[:], in1=gb[:],
                                    op=ALU.mult)
            rstf = _ap(kv, HD, [[2 * HD, NG], [1, HD]])
            nc.vector.tensor_tensor(
                out=rstf, in0=rstf,
                in1=dif[:].rearrange("p (c f) -> p c f", f=HD),
                op=ALU.add)

            # LayerNorm: rst = (rst - mu) * rstd * gamma + beta
            mu = smal.tile([P, NG], F32, tag="sm")
            nc.vector.tensor_reduce(out=mu[:], in_=rstf, axis=AX.X,
                                    op=ALU.add)
            mub = smal.tile([P, NG * HD], F16, tag="sm")
            nc.scalar.activation(
                out=mub[:], in_=_ap(mu[:, 0:1], 0, [[1, NG], [0, HD]]),
                func=ACTF.Copy, scale=1.0 / HD)
            nc.vector.tensor_tensor(
                out=rstf, in0=rstf,
                in1=mub[:].rearrange("p (c f) -> p c f", f=HD),
                op=ALU.subtract)
            sq = smal.tile([P, NG * HD], F16, tag="sm")
            nc.vector.tensor_tensor(
                out=sq[:].rearrange("p (c f) -> p c f", f=HD),
                in0=rstf, in1=rstf, op=ALU.mult)
            vs = smal.tile([P, NG], F32, tag="sm")
            nc.vector.tensor_reduce(
                out=vs[:], in_=sq[:].rearrange("p (c f) -> p c f", f=HD),
                axis=AX.X, op=ALU.add)
            nc.scalar.activation(out=vs[:], in_=vs[:], func=ACTF.Sqrt,
                                 scale=1.0 / HD, bias=eps_t)
            nc.vector.reciprocal(out=vs[:], in_=vs[:])
            rb = smal.tile([P, NG * HD], F16, tag="sm")
            nc.scalar.activation(
                out=rb[:], in_=_ap(vs[:, 0:1], 0, [[1, NG], [0, HD]]),
                func=ACTF.Copy)
            nc.vector.tensor_tensor(
                out=rstf, in0=rstf,
                in1=rb[:].rearrange("p (c f) -> p c f", f=HD),
                op=ALU.mult)
            nc.vector.tensor_tensor(
                out=rstf, in0=rstf,
                in1=_ap(p16[:, 0:1], 2 * HD, [[0, NG], [1, HD]]),
                op=ALU.mult)
            nc.vector.tensor_tensor(
                out=rstf, in0=rstf,
                in1=_ap(p16[:, 0:1], 3 * HD, [[0, NG], [1, HD]]),
                op=ALU.add)
            # PReLU on the rst slots
            nc.scalar.activation(out=rstf, in_=rstf, func=ACTF.Prelu,
                                 alpha=pa)
            nc.sync.dma_start(out=out_d[:], in_=ksk[:])

    nc.compile()
    return nc


# ------------------------------------------------------------------- driver

_CACHE = {}


def _get_nc(plan, ncores):
    key = (tuple(plan["K"].tolist()), plan["grid"], plan["totc"], ncores)
    if key not in _CACHE:
        _CACHE[key] = _build_nc(plan, ncores)
    return _CACHE[key]


def _make_inmaps(plan, params, ncores):
    (Wk, bk, Wskip, bskip, Wgate, bgate, ln_gamma, ln_beta, prelu_a) = params
    Wk = _perm_dh(np.asarray(Wk, np.float32))
    bk = _perm_dh(np.asarray(bk, np.float32).reshape(HD))
    Wskip = _perm_dh(np.asarray(Wskip, np.float32))
    bskip = _perm_dh(np.asarray(bskip, np.float32).reshape(HD))
    wks = np.zeros((IN_F + 1, 2 * HD), np.float16)
    wks[:IN_F, 0:HD] = Wskip
    wks[IN_F, 0:HD] = bskip
    wks[:IN_F, HD:] = Wk
    wks[IN_F, HD:] = bk

    wg = np.asarray(Wgate, np.float32).reshape(3 * HD)
    par16 = np.zeros((1, 4 * HD), np.float16)
    par16[0, 0:HD] = _perm_dh(wg[0:HD] + wg[2 * HD:])          # acts on skip
    par16[0, HD:2 * HD] = _perm_dh(wg[HD:2 * HD] - wg[2 * HD:])  # on rst
    par16[0, 2 * HD:3 * HD] = _perm_dh(np.asarray(ln_gamma, np.float32))
    par16[0, 3 * HD:] = _perm_dh(np.asarray(ln_beta, np.float32))
    par32 = np.zeros((1, 4), np.float32)
    par32[0, 0] = np.float32(np.asarray(bgate).reshape(-1)[0])
    par32[0, 1] = np.float32(np.asarray(prelu_a).reshape(-1)[0])
    par32[0, 2] = 1e-5
    par32[0, 3] = -2.0

    in_maps = []
    for c in range(ncores):
        pc = plan["per_core"][c]
        m = dict(featT=plan["featTs"][c], mask=pc["mask"], tab=pc["tab"],
                 wks=wks, par16=par16, par32=par32)
        in_maps.append(m)
    return in_maps


def run(q_src, v_src, feat, src, dst, Wk, bk, Wskip, bskip, Wgate, bgate,
        ln_gamma, ln_beta, prelu_a, ncores=NCORES, trace=False):
    plan = _plan(q_src, v_src, feat, src, dst, ncores)
    nc = _get_nc(plan, ncores)
    in_maps = _make_inmaps(
        plan, (Wk, bk, Wskip, bskip, Wgate, bgate, ln_gamma, ln_beta, prelu_a),
        ncores)
    res = run_bass_kernel_spmd(nc, in_maps, core_ids=list(range(ncores)),
                               trace=trace)
    n, npc, ngrp = plan["n"], plan["npc"], plan["ngrp"]
    ndum = plan["ndum"]
    out = np.empty((n, HD), np.float32)
    for c in range(ncores):
        r = res.results[c]["out"]                     # [128, ngrp*128] fp16
        rr = r.reshape(P, ngrp, 2, HD)[:, :, 1, :]    # rst slots
        arr = rr.transpose(1, 0, 2).reshape(-1, HD)[ndum:ndum + npc]
        # undo (d, h) interleave -> (h, d)
        arr = arr.reshape(-1, D, H).transpose(0, 2, 1).reshape(-1, HD)
        out[c * npc + plan["cores"][c]["perm"]] = arr
    return out.astype(np.float32), res, plan, in_maps, nc


def kernel(**inputs):
    out, _, _, _, _ = run(**inputs)
    return out


# revision 6
# speedup vs baseline: 2.2234x; 1.1354x over previous
"""Trainium2 Bass kernel for nn_DenTargetTransformerConv (GNN message passing).

Strategy (graph/data parallel, dst-owner sharding across 8 NeuronCores):
  - Nodes are partitioned by dst-id range; each core owns N/8 nodes and all
    edges whose dst falls in its range. Cores are fully independent (the
    "halo exchange" of src features is materialized host-side as per-core
    compacted per-edge tables; the device streams them contiguously).
  - Per core, own nodes are sorted by in-degree and packed into groups of
    128 (SBUF partition dim). Every node in group g gets K[g] edge slots
    (K[g] = max degree in that group position across all cores, so the 8
    cores share one compiled program).
  - All per-edge data lives in fp16 with (d, h)-interleaved head layout so
    every DVE op is a dense step-1 16-bit op (2x perf mode). The edge table
    is stored partition-major in DRAM, so each run is one big contiguous
    dma_start per partition (no gather descriptors).
  - Reductions avoid tensor_reduce (always 1x on DVE) where they are large:
    the D-reduction of scores and the K-reduction of the weighted values are
    log2 trees of 2x tensor_tensor adds. Padded slots carry q=v=0; their
    exp(0-2) contribution to the softmax denominator is removed with a
    host-staged pad-count correction instead of a mask multiply.
  - Runs are software-pipelined (post-exp work of run r issues after the
    pre-exp work of run r+1) so the ACT-engine exp never stalls the DVE.
    The node phase (gate/LayerNorm/PReLU) runs in two interleaved chunks so
    its ACT broadcasts and output DMA overlap DVE work.
"""

import numpy as np

import concourse.bacc as bacc
import concourse.bass as bass
import concourse.tile as tile
from concourse import mybir
from concourse.bass_utils import run_bass_kernel_spmd

F32 = mybir.dt.float32
F16 = mybir.dt.float16
AX = mybir.AxisListType
ALU = mybir.AluOpType
ACTF = mybir.ActivationFunctionType

P = 128
NCORES = 8
HD = 64          # H * D
H, D = 4, 16
IN_F = 64

RUNC = 96        # max slot-columns per merged compute run

# fp16 value the ACT exp produces for a fully-padded slot (exp(0*0.25 - 2))
EXPV = float(np.float32(np.float16(np.exp(-2.0))))


def _perm_dh(m):
    """Permute the last hd axis from (h, d) to (d, h) order."""
    s = m.shape[:-1]
    return m.reshape(*s, H, D).swapaxes(-1, -2).reshape(*s, HD)


# ----------------------------------------------------------------- host prep

def _plan(q_src, v_src, feat, src, dst, ncores):
    n = feat.shape[0]
    npc = n // ncores
    ngrp = (npc + P - 1) // P
    grid = ngrp * P
    ndum = grid - npc

    q2 = _perm_dh(np.asarray(q_src, np.float32).reshape(n, HD))
    v2 = _perm_dh(np.asarray(v_src, np.float32).reshape(n, HD))
    qv = np.concatenate([q2, v2], axis=1).astype(np.float16)   # [n, 128]

    src = np.asarray(src).astype(np.int64)
    dst = np.asarray(dst).astype(np.int64)
    order = np.argsort(dst, kind="stable")
    dst_s, src_s = dst[order], src[order]
    bounds = np.searchsorted(dst_s, np.arange(ncores + 1) * npc)

    cores = []
    gmax = np.zeros((ncores, ngrp), np.int64)
    gdegs = []
    for c in range(ncores):
        lo, hi = bounds[c], bounds[c + 1]
        dstL = dst_s[lo:hi] - c * npc          # ascending
        srcL = src_s[lo:hi]
        deg = np.bincount(dstL, minlength=npc)
        starts = np.concatenate([[0], np.cumsum(deg)])
        rank = np.arange(len(dstL)) - starts[dstL]
        perm = np.argsort(deg, kind="stable")  # ascending degree
        pos_of = np.empty(npc, np.int64)
        pos_of[perm] = ndum + np.arange(npc)
        gd = np.zeros(grid, np.int64)
        gd[ndum:] = deg[perm]
        gmax[c] = gd.reshape(ngrp, P).max(1)
        gdegs.append(gd)
        cores.append(dict(dstL=dstL, srcL=srcL, rank=rank, perm=perm,
                          pos_of=pos_of))

    K = np.maximum(gmax.max(0), 2)             # shared per-group slot count
    colbase = np.concatenate([[0], np.cumsum(K)]).astype(np.int64)
    totc = int(colbase[-1])

    # Per-core edge tables, partition-major: tab[p, col, :] is the qv row of
    # the edge in slot (group g, partition p, rank k), col = colbase[g] + k.
    # negpad removes the padded slots' exp(-2) from the softmax denominator
    # (and folds in the 1e-9 epsilon).
    per_core = []
    for c in range(ncores):
        cd = cores[c]
        pos_e = cd["pos_of"][cd["dstL"]]       # grid position of each edge
        g_e = pos_e // P
        p_e = pos_e % P
        col_e = colbase[g_e] + cd["rank"]
        tab = np.zeros((P, totc, 2 * HD), np.float16)
        tab[p_e, col_e] = qv[cd["srcL"]]
        npad = (K[None, :] - gdegs[c].reshape(ngrp, P).T)      # [P, ngrp]
        negpad = np.repeat((-npad * EXPV + 1e-9).astype(np.float32),
                           H, axis=1)                          # [P, ngrp*H]
        per_core.append(dict(tab=tab.reshape(P, totc * 2 * HD),
                             negpad=negpad))

    # featT with ones row, per core, grid-permuted: [IN_F+1, grid] fp16
    featTs = []
    feat = np.asarray(feat, np.float32)
    for c in range(ncores):
        ft = np.zeros((IN_F + 1, grid), np.float16)
        ft[IN_F, :] = 1.0
        perm = cores[c]["perm"]
        ft[:IN_F, ndum:] = feat[c * npc + perm].T.astype(np.float16)
        featTs.append(ft)

    # Merge consecutive equal-K groups into runs of <= RUNC slot-columns.
    runs = []
    g = 0
    while g < ngrp:
        k = int(K[g])
        ge = g + 1
        while ge < ngrp and int(K[ge]) == k and (ge - g + 1) * k <= RUNC:
            ge += 1
        runs.append((g, ge, k))
        g = ge

    return dict(n=n, npc=npc, ngrp=ngrp, grid=grid, ndum=ndum, K=K,
                colbase=colbase, totc=totc, runs=runs,
                cores=cores, per_core=per_core, featTs=featTs)


# ------------------------------------------------------------- device build

def _ap(view, off, dims):
    """AP over a tile view's buffer: partition dim kept, free dims replaced."""
    return bass.AP(tensor=view.tensor, offset=view.offset + off,
                   ap=[view.ap[0]] + dims)


def _build_nc(plan, ncores):
    ngrp, totc, runs = plan["ngrp"], plan["totc"], plan["runs"]
    grid = plan["grid"]
    colbase = plan["colbase"]
    NG = ngrp

    nc = bacc.Bacc("TRN2", target_bir_lowering=False, debug=False,
                   num_devices=ncores)

    featT_d = nc.dram_tensor("featT", [IN_F + 1, grid], F16,
                             kind="ExternalInput").ap()
    tab_d = nc.dram_tensor("tab", [P, totc * 2 * HD], F16,
                           kind="ExternalInput").ap()
    negpad_d = nc.dram_tensor("negpad", [P, NG * H], F32,
                              kind="ExternalInput").ap()
    # combined node linear weights: cols 0:64 = Wskip|bskip, 64:128 = Wk|bk
    wks_d = nc.dram_tensor("wks", [IN_F + 1, 2 * HD], F16,
                           kind="ExternalInput").ap()
    # fp16 params: [wg_skip (64) | wg_rst (64) | gamma (64) | beta (64)]
    par16_d = nc.dram_tensor("par16", [1, 4 * HD], F16,
                             kind="ExternalInput").ap()
    # fp32 params: [bgate, prelu_a, ln_eps, -2.0]
    par32_d = nc.dram_tensor("par32", [1, 4], F32, kind="ExternalInput").ap()
    out_d = nc.dram_tensor("out", [P, ngrp * 2 * HD], F16,
                           kind="ExternalOutput").ap()

    with tile.TileContext(nc) as tc:
        with (
            tc.tile_pool(name="singles", bufs=1) as singles,
            tc.tile_pool(name="psum", bufs=2, space="PSUM") as psum,
            tc.tile_pool(name="qvp", bufs=3) as qvp,
            tc.tile_pool(name="scr", bufs=4) as scr,
            tc.tile_pool(name="t1p", bufs=2) as t1p,
            tc.tile_pool(name="t2p", bufs=2) as t2p,
            tc.tile_pool(name="t3p", bufs=2) as t3p,
            tc.tile_pool(name="exp", bufs=4) as exsp,
        ):
            # ---- static loads (wks first: matmuls need it + featT chunk)
            wks_sb = singles.tile([IN_F + 1, 2 * HD], F16)
            nc.sync.dma_start(out=wks_sb[:], in_=wks_d[:])
            featT = singles.tile([IN_F + 1, grid], F16)
            FCH = 13 * P
            for f0 in range(0, grid, FCH):
                f1 = min(grid, f0 + FCH)
                nc.sync.dma_start(out=featT[:, f0:f1], in_=featT_d[:, f0:f1])
            p16 = singles.tile([P, 4 * HD], F16)
            nc.gpsimd.dma_start(
                out=p16[:],
                in_=bass.AP(tensor=par16_d.tensor, offset=par16_d.offset,
                            ap=[[0, P], [1, 4 * HD]]))
            p32 = singles.tile([P, 4], F32)
            nc.gpsimd.dma_start(
                out=p32[:],
                in_=bass.AP(tensor=par32_d.tensor, offset=par32_d.offset,
                            ap=[[0, P], [1, 4]]))
            negpad_sb = singles.tile([P, NG * H], F32)
            nc.sync.dma_start(out=negpad_sb[:], in_=negpad_d[:])
            bg = p32[:, 0:1]
            pa = p32[:, 1:2]
            eps_t = p32[:, 2:3]
            nbias = p32[:, 3:4]          # -2.0 shift for exp

            # ksk: per group g, cols [g*128, g*128+64) = skip,
            # [g*128+64, (g+1)*128) = k16 (later overwritten by rst).
            ksk = singles.tile([P, NG * 2 * HD], F16)
            den = singles.tile([P, NG * H], F32)

            # ---- node linears on PE, 4 groups per PSUM bank
            g = 0
            while g < NG:
                nb = min(4, NG - g)
                pk = psum.tile([P, 512], F32, tag="mm")
                for j in range(nb):
                    nc.tensor.matmul(out=pk[:, j * 128:(j + 1) * 128],
                                     lhsT=featT[:, (g + j) * P:(g + j + 1) * P],
                                     rhs=wks_sb[:],
                                     start=True, stop=True)
                nc.scalar.activation(out=ksk[:, g * 128:(g + nb) * 128],
                                     in_=pk[:, :nb * 128], func=ACTF.Copy)
                g += nb

            # ---- edge phase, software-pipelined over runs
            def emit_pre(g0, g1, K):
                R = g1 - g0
                RK = R * K
                c0 = int(colbase[g0])
                qv_t = qvp.tile([P, RUNC * 2 * HD], F16, tag="qv")
                nc.sync.dma_start(out=qv_t[:, :RK * 2 * HD],
                                  in_=tab_d[:, c0 * 2 * HD:(c0 + RK) * 2 * HD])
                qv0 = qv_t[:, 0:1]

                # prod[p, (r,k), dh] = q * k_dst  (k16 bcast over k slots)
                prod = scr.tile([P, RUNC * HD], F16, tag="scr")
                pr0 = prod[:, 0:1]
                nc.vector.tensor_tensor(
                    out=_ap(pr0, 0, [[HD * K, R], [HD, K], [1, HD]]),
                    in0=_ap(qv0, 0, [[2 * HD * K, R], [2 * HD, K], [1, HD]]),
                    in1=_ap(ksk[:, 0:1], g0 * 2 * HD + HD,
                            [[2 * HD, R], [0, K], [1, HD]]),
                    op=ALU.mult)

                # score tree over d: prod [p, rk, d16, h] -> a [p, rk, h]
                t1 = t1p.tile([P, RUNC * 32], F16, tag="t1")
                nc.vector.tensor_tensor(
                    out=_ap(t1[:, 0:1], 0, [[32, RK], [H, 8], [1, H]]),
                    in0=_ap(pr0, 0, [[HD, RK], [H, 8], [1, H]]),
                    in1=_ap(pr0, 32, [[HD, RK], [H, 8], [1, H]]),
                    op=ALU.add)
                t2 = t2p.tile([P, RUNC * 16], F16, tag="t2")
                nc.vector.tensor_tensor(
                    out=_ap(t2[:, 0:1], 0, [[16, RK], [H, 4], [1, H]]),
                    in0=_ap(t1[:, 0:1], 0, [[32, RK], [H, 4], [1, H]]),
                    in1=_ap(t1[:, 0:1], 16, [[32, RK], [H, 4], [1, H]]),
                    op=ALU.add)
                t3 = t3p.tile([P, RUNC * 8], F16, tag="t3")
                nc.vector.tensor_tensor(
                    out=_ap(t3[:, 0:1], 0, [[8, RK], [H, 2], [1, H]]),
                    in0=_ap(t2[:, 0:1], 0, [[16, RK], [H, 2], [1, H]]),
                    in1=_ap(t2[:, 0:1], 8, [[16, RK], [H, 2], [1, H]]),
                    op=ALU.add)
                ex = exsp.tile([P, RUNC * H], F16, tag="ex")
                nc.vector.tensor_tensor(
                    out=_ap(ex[:, 0:1], 0, [[H, RK], [1, H]]),
                    in0=_ap(t3[:, 0:1], 0, [[2 * H, RK], [1, H]]),
                    in1=_ap(t3[:, 0:1], H, [[2 * H, RK], [1, H]]),
                    op=ALU.add)

                # ex = exp(a/4 - 2) (ACT; padded slots give exp(-2), removed
                # from the denominator via negpad)
                exf = ex[:, :RK * H]
                nc.scalar.activation(out=exf, in_=exf, func=ACTF.Exp,
                                     scale=0.25, bias=nbias)
                return qv_t, ex

            def emit_post(g0, g1, K, qv_t, ex):
                R = g1 - g0
                RK = R * K
                qv0 = qv_t[:, 0:1]
                # denom[p, r, h] = sum_k ex
                nc.vector.tensor_reduce(
                    out=_ap(den[:, 0:1], g0 * H, [[H, R], [1, H]]),
                    in_=_ap(ex[:, 0:1], 0, [[K * H, R], [1, H], [H, K]]),
                    axis=AX.X, op=ALU.add)

                # w[p, rk, d, h] = v * ex (bcast over d)
                w_t = scr.tile([P, RUNC * HD], F16, tag="scr")
                w0 = w_t[:, 0:1]
                nc.vector.tensor_tensor(
                    out=_ap(w0, 0, [[HD, RK], [H, D], [1, H]]),
                    in0=_ap(qv0, HD, [[2 * HD, RK], [H, D], [1, H]]),
                    in1=_ap(ex[:, 0:1], 0, [[H, RK], [0, D], [1, H]]),
                    op=ALU.mult)

                # agg tree over k -> rst slot of ksk (fp16)
                klen = K
                while klen > 2:
                    h1 = klen // 2
                    nc.vector.tensor_tensor(
                        out=_ap(w0, 0, [[K * HD, R], [HD, h1], [1, HD]]),
                        in0=_ap(w0, 0, [[K * HD, R], [HD, h1], [1, HD]]),
                        in1=_ap(w0, (klen - h1) * HD,
                                [[K * HD, R], [HD, h1], [1, HD]]),
                        op=ALU.add)
                    klen = h1 + (klen & 1)
                nc.vector.tensor_tensor(
                    out=_ap(ksk[:, 0:1], g0 * 2 * HD + HD,
                            [[2 * HD, R], [1, HD]]),
                    in0=_ap(w0, 0, [[K * HD, R], [1, HD]]),
                    in1=_ap(w0, HD, [[K * HD, R], [1, HD]]),
                    op=ALU.add)

            pend = None
            for (g0, g1, K) in runs:
                pre = emit_pre(g0, g1, K)
                if pend is not None:
                    emit_post(*pend)
                pend = (g0, g1, K) + pre
            emit_post(*pend)

            # ---- node phase: two interleaved chunks of groups
            kv = ksk[:, 0:1]

            def node_ops(lo, hi):
                NGc = hi - lo
                dsl = den[:, lo * H:hi * H]
                nsl = negpad_sb[:, lo * H:hi * H]
                d16 = singles.tile([P, NGc * H], F16)
                gl = singles.tile([P, NGc], F32)
                g16 = singles.tile([P, NGc], F16)
                gb = singles.tile([P, NGc * HD], F16)
                dif = singles.tile([P, NGc * HD], F16)
                mu = singles.tile([P, NGc], F32)
                mub = singles.tile([P, NGc * HD], F16)
                sq = singles.tile([P, NGc * HD], F16)
                vs = singles.tile([P, NGc], F32)
                rb = singles.tile([P, NGc * HD], F16)
                rstf = _ap(kv, lo * 2 * HD + HD, [[2 * HD, NGc], [1, HD]])
                sksl = _ap(kv, lo * 2 * HD, [[2 * HD, NGc], [1, HD]])
                dif3 = dif[:].rearrange("p (c f) -> p c f", f=HD)
                mub3 = mub[:].rearrange("p (c f) -> p c f", f=HD)
                sq3 = sq[:].rearrange("p (c f) -> p c f", f=HD)
                rb3 = rb[:].rearrange("p (c f) -> p c f", f=HD)
                zt = qvp.tile([P, RUNC * 2 * HD], F16, tag="qv")
                ops = [
                    # dinv = 1 / (den - npad*e^-2 + 1e-9), as fp16
                    lambda: nc.vector.tensor_tensor(
                        out=dsl, in0=dsl, in1=nsl, op=ALU.add),
                    lambda: nc.vector.reciprocal(out=dsl, in_=dsl),
                    lambda: nc.scalar.activation(out=d16[:], in_=dsl,
                                                 func=ACTF.Copy),
                    # rst = agg * dinv
                    lambda: nc.vector.tensor_tensor(
                        out=_ap(kv, lo * 2 * HD + HD,
                                [[2 * HD, NGc], [H, D], [1, H]]),
                        in0=_ap(kv, lo * 2 * HD + HD,
                                [[2 * HD, NGc], [H, D], [1, H]]),
                        in1=_ap(d16[:, 0:1], 0, [[H, NGc], [0, D], [1, H]]),
                        op=ALU.mult),
                    # gate logit z = sum over 128 of [skip|rst]*[wgs|wgr]
                    lambda: nc.vector.tensor_tensor(
                        out=_ap(zt[:, 0:1], 0, [[2 * HD, NGc], [1, 2 * HD]]),
                        in0=_ap(kv, lo * 2 * HD, [[2 * HD, NGc], [1, 2 * HD]]),
                        in1=_ap(p16[:, 0:1], 0, [[0, NGc], [1, 2 * HD]]),
                        op=ALU.mult),
                    lambda: nc.vector.tensor_reduce(
                        out=gl[:],
                        in_=_ap(zt[:, 0:1], 0, [[2 * HD, NGc], [1, 2 * HD]]),
                        axis=AX.X, op=ALU.add),
                    lambda: nc.scalar.activation(out=g16[:], in_=gl[:],
                                                 func=ACTF.Sigmoid, bias=bg),
                    lambda: nc.scalar.activation(
                        out=gb[:],
                        in_=_ap(g16[:, 0:1], 0, [[1, NGc], [0, HD]]),
                        func=ACTF.Copy),
                    # rst += gate * (skip - rst)
                    lambda: nc.vector.tensor_tensor(
                        out=dif3, in0=sksl, in1=rstf, op=ALU.subtract),
                    lambda: nc.vector.tensor_tensor(
                        out=dif[:], in0=dif[:], in1=gb[:], op=ALU.mult),
                    lambda: nc.vector.tensor_tensor(
                        out=rstf, in0=rstf, in1=dif3, op=ALU.add),
                    # LayerNorm
                    lambda: nc.vector.tensor_reduce(
                        out=mu[:], in_=rstf, axis=AX.X, op=ALU.add),
                    lambda: nc.scalar.activation(
                        out=mub[:],
                        in_=_ap(mu[:, 0:1], 0, [[1, NGc], [0, HD]]),
                        func=ACTF.Copy, scale=1.0 / HD),
                    lambda: nc.vector.tensor_tensor(
                        out=rstf, in0=rstf, in1=mub3, op=ALU.subtract),
                    lambda: nc.vector.tensor_tensor(
                        out=sq3, in0=rstf, in1=rstf, op=ALU.mult),
                    lambda: nc.vector.tensor_reduce(
                        out=vs[:], in_=sq3, axis=AX.X, op=ALU.add),
                    lambda: nc.scalar.activation(out=vs[:], in_=vs[:],
                                                 func=ACTF.Sqrt,
                                                 scale=1.0 / HD, bias=eps_t),
                    lambda: nc.vector.reciprocal(out=vs[:], in_=vs[:]),
                    lambda: nc.scalar.activation(
                        out=rb[:],
                        in_=_ap(vs[:, 0:1], 0, [[1, NGc], [0, HD]]),
                        func=ACTF.Copy),
                    lambda: nc.vector.tensor_tensor(
                        out=rstf, in0=rstf, in1=rb3, op=ALU.mult),
                    lambda: nc.vector.tensor_tensor(
                        out=rstf, in0=rstf,
                        in1=_ap(p16[:, 0:1], 2 * HD, [[0, NGc], [1, HD]]),
                        op=ALU.mult),
                    lambda: nc.vector.tensor_tensor(
                        out=rstf, in0=rstf,
                        in1=_ap(p16[:, 0:1], 3 * HD, [[0, NGc], [1, HD]]),
                        op=ALU.add),
                    lambda: nc.scalar.activation(out=rstf, in_=rstf,
                                                 func=ACTF.Prelu, alpha=pa),
                    lambda: nc.sync.dma_start(
                        out=out_d[:, lo * 2 * HD:hi * 2 * HD],
                        in_=ksk[:, lo * 2 * HD:hi * 2 * HD]),
                ]
                return ops

            mid = NG // 2
            opsA = node_ops(0, mid)
            opsB = node_ops(mid, NG)
            for a, b in zip(opsA, opsB):
                a()
                b()

    nc.compile()
    return nc


# ------------------------------------------------------------------- driver

_CACHE = {}


def _get_nc(plan, ncores):
    key = (tuple(plan["K"].tolist()), plan["grid"], plan["totc"], ncores)
    if key not in _CACHE:
        _CACHE[key] = _build_nc(plan, ncores)
    return _CACHE[key]


def _make_inmaps(plan, params, ncores):
    (Wk, bk, Wskip, bskip, Wgate, bgate, ln_gamma, ln_beta, prelu_a) = params
    Wk = _perm_dh(np.asarray(Wk, np.float32))
    bk = _perm_dh(np.asarray(bk, np.float32).reshape(HD))
    Wskip = _perm_dh(np.asarray(Wskip, np.float32))
    bskip = _perm_dh(np.asarray(bskip, np.float32).reshape(HD))
    wks = np.zeros((IN_F + 1, 2 * HD), np.float16)
    wks[:IN_F, 0:HD] = Wskip
    wks[IN_F, 0:HD] = bskip
    wks[:IN_F, HD:] = Wk
    wks[IN_F, HD:] = bk

    wg = np.asarray(Wgate, np.float32).reshape(3 * HD)
    par16 = np.zeros((1, 4 * HD), np.float16)
    par16[0, 0:HD] = _perm_dh(wg[0:HD] + wg[2 * HD:])          # acts on skip
    par16[0, HD:2 * HD] = _perm_dh(wg[HD:2 * HD] - wg[2 * HD:])  # on rst
    par16[0, 2 * HD:3 * HD] = _perm_dh(np.asarray(ln_gamma, np.float32))
    par16[0, 3 * HD:] = _perm_dh(np.asarray(ln_beta, np.float32))
    par32 = np.zeros((1, 4), np.float32)
    par32[0, 0] = np.float32(np.asarray(bgate).reshape(-1)[0])
    par32[0, 1] = np.float32(np.asarray(prelu_a).reshape(-1)[0])
    par32[0, 2] = 1e-5
    par32[0, 3] = -2.0

    in_maps = []
    for c in range(ncores):
        pc = plan["per_core"][c]
        m = dict(featT=plan["featTs"][c], negpad=pc["negpad"], tab=pc["tab"],
                 wks=wks, par16=par16, par32=par32)
        in_maps.append(m)
    return in_maps


def run(q_src, v_src, feat, src, dst, Wk, bk, Wskip, bskip, Wgate, bgate,
        ln_gamma, ln_beta, prelu_a, ncores=NCORES, trace=False):
    plan = _plan(q_src, v_src, feat, src, dst, ncores)
    nc = _get_nc(plan, ncores)
    in_maps = _make_inmaps(
        plan, (Wk, bk, Wskip, bskip, Wgate, bgate, ln_gamma, ln_beta, prelu_a),
        ncores)
    res = run_bass_kernel_spmd(nc, in_maps, core_ids=list(range(ncores)),
                               trace=trace)
    n, npc, ngrp = plan["n"], plan["npc"], plan["ngrp"]
    ndum = plan["ndum"]
    out = np.empty((n, HD), np.float32)
    for c in range(ncores):
        r = res.results[c]["out"]                     # [128, ngrp*128] fp16
        rr = r.reshape(P, ngrp, 2, HD)[:, :, 1, :]    # rst slots
        arr = rr.transpose(1, 0, 2).reshape(-1, HD)[ndum:ndum + npc]
        # undo (d, h) interleave -> (h, d)
        arr = arr.reshape(-1, D, H).transpose(0, 2, 1).reshape(-1, HD)
        out[c * npc + plan["cores"][c]["perm"]] = arr
    return out.astype(np.float32), res, plan, in_maps, nc


def kernel(**inputs):
    out, _, _, _, _ = run(**inputs)
    return out
